# revision 7
# baseline (speedup 1.0000x reference)
"""Causal MHA (AdaLNAttention) on 8 TRN2 NeuronCores.

Sharding: tensor-parallel on heads (2 heads/core) for QKV projections and
attention (fully local, perfectly balanced since every core does the full
causal triangle for its heads), then one small AllToAll re-shards from
head-split to row-split, and the output projection runs row-parallel
(256 rows/core) with no reduction.

All matmuls run in bf16 (f32 PSUM accumulate); softmax exp in f32 on ACT.
Host-side work is layout/dtype only: transposes, slicing, bf16 casts.
"""

import math

import numpy as np
import ml_dtypes

S = 2048
D = 1024
H = 16
HD = 64
NCORES = 8
RPC = S // NCORES  # 256 rows per core after A2A

_CACHE = {}


def _build():
    import concourse.bass as bass
    import concourse.mybir as mybir
    import concourse.tile as tile
    from concourse import bacc
    from concourse.masks import make_identity

    BF16 = mybir.dt.bfloat16
    F32 = mybir.dt.float32
    Exp = mybir.ActivationFunctionType.Exp

    nc = bacc.Bacc(
        "TRN2", target_bir_lowering=False, debug=False, num_devices=NCORES
    )

    qt_d = nc.declare_dram_parameter("qt", [D, S], BF16, isOutput=False)
    kt_d = nc.declare_dram_parameter("kt", [D, S], BF16, isOutput=False)
    vt_d = nc.declare_dram_parameter("vt", [D, S], BF16, isOutput=False)
    wq_d = nc.declare_dram_parameter("wq", [D, 128], BF16, isOutput=False)
    wk_d = nc.declare_dram_parameter("wk", [D, 128], BF16, isOutput=False)
    wv_d = nc.declare_dram_parameter("wv", [D, 128], BF16, isOutput=False)
    wo_d = nc.declare_dram_parameter("wo", [D, D], BF16, isOutput=False)
    bq_d = nc.declare_dram_parameter("bq", [128, 1], F32, isOutput=False)
    bk_d = nc.declare_dram_parameter("bk", [128, 1], F32, isOutput=False)
    bv_d = nc.declare_dram_parameter("bv", [128, 1], F32, isOutput=False)
    bo_d = nc.declare_dram_parameter("bo", [128, 8], F32, isOutput=False)
    mask_d = nc.declare_dram_parameter("mask", [128, 128], BF16, isOutput=False)
    out_d = nc.declare_dram_parameter("out", [D, RPC], F32, isOutput=True)

    with tile.TileContext(nc) as tc:
        with (
            tc.tile_pool(name="cpool", bufs=1) as cpool,
            tc.tile_pool(name="wpool", bufs=1) as wpool,
            tc.tile_pool(name="spool", bufs=1) as spool,
            tc.tile_pool(name="chunks", bufs=4) as chpool,
            tc.tile_pool(name="probsp", bufs=4) as prpool,
            tc.tile_pool(name="accp", bufs=4) as accpool,
            tc.tile_pool(name="big", bufs=2, space="PSUM") as big,
            tc.tile_pool(name="pa", bufs=2, space="PSUM") as pa,
            tc.tile_pool(name="tiny", bufs=2, space="PSUM") as tiny,
            tc.tile_pool(name="dram", bufs=1, space="DRAM") as dram,
        ):
            # ---- constants / small inputs ----
            mask_sb = cpool.tile([128, 128], BF16)
            nc.sync.dma_start(out=mask_sb, in_=mask_d[:, :])
            bq_sb = cpool.tile([128, 1], F32)
            nc.sync.dma_start(out=bq_sb, in_=bq_d[:, :])
            bk_sb = cpool.tile([128, 1], F32)
            nc.sync.dma_start(out=bk_sb, in_=bk_d[:, :])
            bv_sb = cpool.tile([128, 1], F32)
            nc.sync.dma_start(out=bv_sb, in_=bv_d[:, :])
            bo_sb = cpool.tile([128, 8], F32)
            nc.sync.dma_start(out=bo_sb, in_=bo_d[:, :])
            ones_col = cpool.tile([128, 1], F32)
            nc.vector.memset(ones_col, 1.0)
            ones_row = cpool.tile([1, 128], BF16)
            nc.vector.memset(ones_row, 1.0)
            ident = cpool.tile([128, 128], BF16)
            make_identity(nc, ident)

            # ---- weights ----
            wq_sb = wpool.tile([128, 8, 128], BF16)
            nc.sync.dma_start(out=wq_sb, in_=wq_d.rearrange("(dc p) m -> p dc m", p=128))
            wk_sb = wpool.tile([128, 8, 128], BF16)
            nc.sync.dma_start(out=wk_sb, in_=wk_d.rearrange("(dc p) m -> p dc m", p=128))
            wv_sb = wpool.tile([128, 8, 128], BF16)
            nc.sync.dma_start(out=wv_sb, in_=wv_d.rearrange("(dc p) m -> p dc m", p=128))
            wo_sb = wpool.tile([128, 8, 1024], BF16)
            nc.sync.dma_start(out=wo_sb, in_=wo_d.rearrange("(i p) m -> p i m", p=128))

            # ---- persistent activations ----
            qT_sb = spool.tile([128, S], BF16)   # [2 heads x 64, 2048 rows]
            kT_sb = spool.tile([128, S], BF16)
            vT_sb = spool.tile([128, S], BF16)
            v_sb = spool.tile([128, 16, 128], BF16)   # [keys-in-chunk, kc, 2 heads x 64]
            attnT_sb = spool.tile([128, 16, 128], BF16)  # [2 heads x 64, block, q-in-block]
            a2a_sb = spool.tile([128, 8, 256], BF16)
            outT_sb = spool.tile([128, 8, 256], F32)

            # ---- projections: out.T[m, r] = sum_d WT[d, m] * XT[d, r] ----
            def proj(in_d, w_sb, dst_sb, bias_sb):
                p0 = big.tile([128, 1024], F32, tag="big", name="p0")
                p1 = big.tile([128, 1024], F32, tag="big", name="p1")
                slices = [(p0, 0), (p0, 512), (p1, 0), (p1, 512)]
                for dc in range(8):
                    ch = chpool.tile([128, S], BF16, tag="chunk", name="ch")
                    nc.sync.dma_start(out=ch, in_=in_d[dc * 128:(dc + 1) * 128, :])
                    for rg in range(4):
                        pt, off = slices[rg]
                        nc.tensor.matmul(
                            pt[:, off:off + 512],
                            lhsT=w_sb[:, dc, :],
                            rhs=ch[:, rg * 512:(rg + 1) * 512],
                            start=(dc == 0),
                            stop=(dc == 7),
                        )
                for rg in range(4):
                    pt, off = slices[rg]
                    if bias_sb is not None:
                        nc.vector.tensor_scalar_add(
                            dst_sb[:, rg * 512:(rg + 1) * 512],
                            pt[:, off:off + 512],
                            bias_sb[:, 0:1],
                        )
                    else:
                        nc.vector.tensor_copy(
                            dst_sb[:, rg * 512:(rg + 1) * 512], pt[:, off:off + 512]
                        )

            proj(kt_d, wk_sb, kT_sb, bk_sb)
            proj(qt_d, wq_sb, qT_sb, bq_sb)
            proj(vt_d, wv_sb, vT_sb, None)  # bv folded in after attnV

            # vT -> v natural via PE transpose
            for rt in range(16):
                tp = pa.tile([128, 128], BF16, tag="pa", name="tp")
                nc.tensor.transpose(tp, vT_sb[:, rt * 128:(rt + 1) * 128], ident)
                nc.vector.tensor_copy(v_sb[:, rt, :], tp)

            # ---- attention (2 heads, full causal triangle) ----
            for b in range(16):
                pat = pa.tile([128, 128], F32, tag="pa", name="pat")
                for h in range(2):
                    hp = slice(h * 64, (h + 1) * 64)
                    nkc = b + 1
                    probs = prpool.tile([128, nkc * 128], BF16, tag="probs", name="probs")
                    ngroups = (nkc + 7) // 8
                    for g in range(ngroups):
                        nk_g = min(8, nkc - g * 8)
                        ps = big.tile([128, 1024], F32, tag="big", name="ps")
                        for kci in range(nk_g):
                            kc = g * 8 + kci
                            nc.tensor.matmul(
                                ps[:, kci * 128:(kci + 1) * 128],
                                lhsT=kT_sb[hp, kc * 128:(kc + 1) * 128],
                                rhs=qT_sb[hp, b * 128:(b + 1) * 128],
                                start=True,
                                stop=True,
                            )
                        nc.scalar.activation(
                            probs[:, g * 1024:g * 1024 + nk_g * 128],
                            ps[:, 0:nk_g * 128],
                            Exp,
                            scale=0.125,
                        )
                    # causal mask on the diagonal chunk
                    nc.vector.tensor_mul(
                        probs[:, b * 128:(b + 1) * 128],
                        probs[:, b * 128:(b + 1) * 128],
                        mask_sb,
                    )
                    # denominator: sum over kc (strided free reduce), then over
                    # partitions (ones matmul)
                    acc = accpool.tile([128, 128], F32, tag="acc", name="acc")
                    pr3 = probs.rearrange("p (kc i) -> p i kc", i=128)
                    nc.vector.tensor_reduce(
                        acc.rearrange("p (i one) -> p i one", one=1),
                        pr3,
                        axis=mybir.AxisListType.X,
                        op=mybir.AluOpType.add,
                    )
                    pdt = tiny.tile([128, 128], F32, tag="tiny", name="pdt")
                    nc.tensor.matmul(
                        pdt[0:1, :], lhsT=ones_col, rhs=acc, start=True, stop=True
                    )
                    recip = accpool.tile([1, 128], BF16, tag="recip", name="recip")
                    with nc.allow_low_precision(reason="bf16 softmax recip ok at 2e-2 gate"):
                        nc.vector.reciprocal(recip, pdt[0:1, :])
                    pbt = tiny.tile([128, 128], F32, tag="tiny", name="pbt")
                    nc.tensor.matmul(
                        pbt, lhsT=ones_row, rhs=recip, start=True, stop=True
                    )
                    bc = accpool.tile([128, 128], BF16, tag="bc", name="bc")
                    nc.vector.tensor_copy(bc, pbt)
                    # normalize probs (broadcast recip over kc)
                    pr3n = probs.rearrange("p (kc i) -> p kc i", i=128)
                    bc3 = bc.rearrange("p (one i) -> p one i", one=1).broadcast_to(
                        [128, nkc, 128]
                    )
                    nc.vector.tensor_mul(pr3n, pr3n, bc3)
                    # attnV accumulate over kc -> attn.T [64, 128]
                    for kc in range(nkc):
                        nc.tensor.matmul(
                            pat[hp, :],
                            lhsT=v_sb[:, kc, hp],
                            rhs=probs[:, kc * 128:(kc + 1) * 128],
                            start=(kc == 0),
                            stop=(kc == nkc - 1),
                            tile_position=(0, h * 64),
                            skip_group_check=True,
                        )
                # evacuate both heads with bv bias
                nc.vector.tensor_scalar_add(attnT_sb[:, b, :], pat, bv_sb[:, 0:1])

            # ---- AllToAll: head-split -> row-split ----
            a2a_in = dram.tile([1024, 256], BF16, name="a2a_in")
            a2a_out = dram.tile([1024, 256], BF16, name="a2a_out")
            for j in range(8):
                nc.sync.dma_start(
                    out=a2a_in[j * 128:(j + 1) * 128, :],
                    in_=attnT_sb[:, 2 * j:2 * j + 2, :],
                )
            nc.gpsimd.collective_compute(
                "AllToAll",
                mybir.AluOpType.bypass,
                ins=[a2a_in.opt()],
                outs=[a2a_out.opt()],
                replica_groups=[list(range(NCORES))],
            )
            nc.sync.dma_start(
                out=a2a_sb, in_=a2a_out.rearrange("(i p) r -> p i r", p=128)
            )

            # ---- output projection: out.T[m, r] = sum_n WoT[n, m] attnT[n, r] ----
            for t in range(8):
                po = pa.tile([128, 256], F32, tag="pa", name="po")
                for i in range(8):
                    nc.tensor.matmul(
                        po,
                        lhsT=wo_sb[:, i, t * 128:(t + 1) * 128],
                        rhs=a2a_sb[:, i, :],
                        start=(i == 0),
                        stop=(i == 7),
                    )
                nc.vector.tensor_scalar_add(outT_sb[:, t, :], po, bo_sb[:, t:t + 1])
                nc.sync.dma_start(
                    out=out_d[t * 128:(t + 1) * 128, :], in_=outT_sb[:, t, :]
                )

    nc.finalize()
    return nc


def _get_nc():
    if "nc" not in _CACHE:
        _CACHE["nc"] = _build()
    return _CACHE["nc"]


def kernel(Q, K, V, Wq, bq, Wk, bk, Wv, bv, Wo, bo):
    from concourse.bass_utils import run_bass_kernel_spmd

    bf16 = ml_dtypes.bfloat16
    f32 = np.float32

    QT = np.ascontiguousarray(np.asarray(Q).T).astype(bf16)  # [D, S]
    KT = np.ascontiguousarray(np.asarray(K).T).astype(bf16)
    VT = np.ascontiguousarray(np.asarray(V).T).astype(bf16)
    WqT = np.ascontiguousarray(np.asarray(Wq).T).astype(bf16)  # [D, D] = [d, m]
    WkT = np.ascontiguousarray(np.asarray(Wk).T).astype(bf16)
    WvT = np.ascontiguousarray(np.asarray(Wv).T).astype(bf16)
    WoT = np.ascontiguousarray(np.asarray(Wo).T).astype(bf16)  # [n, m]
    bq = np.asarray(bq, f32)
    bk = np.asarray(bk, f32)
    bv = np.asarray(bv, f32)
    bo = np.asarray(bo, f32)
    bo_t = np.ascontiguousarray(bo.reshape(8, 128).T).astype(f32)  # [128, 8]
    mask = np.ascontiguousarray(
        np.triu(np.ones((128, 128), dtype=np.float32))
    ).astype(bf16)  # keep key j <= query i in [j, i] layout

    in_maps = []
    for c in range(NCORES):
        hs = slice(c * 128, (c + 1) * 128)  # this core's 2 heads' proj dims
        in_maps.append(
            {
                "qt": QT,
                "kt": KT,
                "vt": VT,
                "wq": np.ascontiguousarray(WqT[:, hs]),
                "wk": np.ascontiguousarray(WkT[:, hs]),
                "wv": np.ascontiguousarray(WvT[:, hs]),
                "wo": WoT,
                "bq": np.ascontiguousarray(bq[hs].reshape(128, 1)),
                "bk": np.ascontiguousarray(bk[hs].reshape(128, 1)),
                "bv": np.ascontiguousarray(bv[hs].reshape(128, 1)),
                "bo": bo_t,
                "mask": mask,
            }
        )

    nc = _get_nc()
    res = run_bass_kernel_spmd(nc, in_maps, core_ids=list(range(NCORES)))
    _CACHE["last_results"] = res
    outs = [r["out"] for r in res.results]  # each [D, RPC] f32 = out.T rows chunk
    final = np.concatenate([o.T for o in outs], axis=0)  # [S, D]
    return final.astype(np.float32)


# revision 16
# speedup vs baseline: 1.1653x; 1.1653x over previous
"""Causal MHA (AdaLNAttention) on 8 TRN2 NeuronCores.

Sharding: tensor-parallel on heads (2 heads/core) for QKV projections and
attention (fully local, perfectly balanced since every core does the full
causal triangle for its heads), then one small AllToAll re-shards from
head-split to row-split, and the output projection runs row-parallel
(256 rows/core) with no reduction.

All matmuls run in bf16 (f32 PSUM accumulate); softmax exp in f32 on ACT.
Host-side work is layout/dtype only: transposes, slicing, bf16 casts.
"""

import math

import numpy as np
import ml_dtypes

S = 2048
D = 1024
H = 16
HD = 64
NCORES = 8
RPC = S // NCORES  # 256 rows per core after A2A

_CACHE = {}


def _build():
    import concourse.bass as bass
    import concourse.mybir as mybir
    import concourse.tile as tile
    from concourse import bacc
    from concourse.masks import make_identity

    BF16 = mybir.dt.bfloat16
    F32 = mybir.dt.float32
    Exp = mybir.ActivationFunctionType.Exp

    nc = bacc.Bacc(
        "TRN2", target_bir_lowering=False, debug=False, num_devices=NCORES
    )

    qt_d = nc.declare_dram_parameter("qt", [D, S], BF16, isOutput=False)
    kt_d = nc.declare_dram_parameter("kt", [D, S], BF16, isOutput=False)
    vt_d = nc.declare_dram_parameter("vt", [D, S], BF16, isOutput=False)
    wq_d = nc.declare_dram_parameter("wq", [D, 128], BF16, isOutput=False)
    wk_d = nc.declare_dram_parameter("wk", [D, 128], BF16, isOutput=False)
    wv_d = nc.declare_dram_parameter("wv", [D, 128], BF16, isOutput=False)
    wo_d = nc.declare_dram_parameter("wo", [D, D], BF16, isOutput=False)
    bq_d = nc.declare_dram_parameter("bq", [128, 1], F32, isOutput=False)
    bk_d = nc.declare_dram_parameter("bk", [128, 1], F32, isOutput=False)
    bv_d = nc.declare_dram_parameter("bv", [128, 8], F32, isOutput=False)
    bo_d = nc.declare_dram_parameter("bo", [128, 8], F32, isOutput=False)
    mask_d = nc.declare_dram_parameter("mask", [128, 128], BF16, isOutput=False)
    out_d = nc.declare_dram_parameter("out", [D, RPC], F32, isOutput=True)

    with tile.TileContext(nc) as tc:
        with (
            tc.tile_pool(name="cpool", bufs=1) as cpool,
            tc.tile_pool(name="wpool", bufs=1) as wpool,
            tc.tile_pool(name="spool", bufs=1) as spool,
            tc.tile_pool(name="chunks", bufs=4) as chpool,
            tc.tile_pool(name="probsp", bufs=4) as prpool,
            tc.tile_pool(name="accp", bufs=4) as accpool,
            tc.tile_pool(name="big", bufs=3, space="PSUM") as big,
            tc.tile_pool(name="pa", bufs=2, space="PSUM") as pa,
            tc.tile_pool(name="dram", bufs=1, space="DRAM") as dram,
        ):
            # ---- constants / small inputs ----
            mask_sb = cpool.tile([128, 128], BF16)
            nc.sync.dma_start(out=mask_sb, in_=mask_d[:, :])
            bq_sb = cpool.tile([128, 1], F32)
            nc.sync.dma_start(out=bq_sb, in_=bq_d[:, :])
            bk_sb = cpool.tile([128, 1], F32)
            nc.sync.dma_start(out=bk_sb, in_=bk_d[:, :])
            bv_sb = cpool.tile([128, 8], F32)
            nc.sync.dma_start(out=bv_sb, in_=bv_d[:, :])
            bo_sb = cpool.tile([128, 8], F32)
            nc.sync.dma_start(out=bo_sb, in_=bo_d[:, :])
            ones_row = cpool.tile([1, 128], BF16)
            nc.vector.memset(ones_row, 1.0)
            ident = cpool.tile([128, 128], BF16)
            make_identity(nc, ident)

            # ---- weights ----
            wq_sb = wpool.tile([128, 8, 128], BF16)
            nc.sync.dma_start(out=wq_sb, in_=wq_d.rearrange("(dc p) m -> p dc m", p=128))
            wk_sb = wpool.tile([128, 8, 128], BF16)
            nc.sync.dma_start(out=wk_sb, in_=wk_d.rearrange("(dc p) m -> p dc m", p=128))
            wv_sb = wpool.tile([128, 8, 128], BF16)
            nc.sync.dma_start(out=wv_sb, in_=wv_d.rearrange("(dc p) m -> p dc m", p=128))
            wo_sb = wpool.tile([128, 8, 1024], BF16)
            nc.sync.dma_start(out=wo_sb, in_=wo_d.rearrange("(i p) m -> p i m", p=128))

            # ---- persistent activations ----
            qT_sb = spool.tile([128, S], BF16)   # [2 heads x 64, 2048 rows]
            kT_sb = spool.tile([128, S], BF16)
            vT_sb = spool.tile([128, S], BF16)
            # [keys-in-chunk, kc, (v_h0 | 1 | v_h1 | 1)] — ones col rides the
            # attnV matmul to produce the softmax denominator for free
            vaug_sb = spool.tile([128, 16, 130], BF16)
            attnT_sb = spool.tile([128, 16, 128], BF16)  # [2 heads x 64, block, q-in-block]
            a2a_sb = spool.tile([128, 8, 256], BF16)
            outT_sb = spool.tile([128, 8, 256], F32)

            # ---- projections: out.T[m, r] = sum_d WT[d, m] * XT[d, r] ----
            def proj(in_d, w_sb, dst_sb, bias_sb):
                p0 = big.tile([128, 1024], F32, tag="big", name="p0")
                p1 = big.tile([128, 1024], F32, tag="big", name="p1")
                slices = [(p0, 0), (p0, 512), (p1, 0), (p1, 512)]
                for dc in range(8):
                    ch = chpool.tile([128, S], BF16, tag="chunk", name="ch")
                    nc.sync.dma_start(out=ch, in_=in_d[dc * 128:(dc + 1) * 128, :])
                    for rg in range(4):
                        pt, off = slices[rg]
                        nc.tensor.matmul(
                            pt[:, off:off + 512],
                            lhsT=w_sb[:, dc, :],
                            rhs=ch[:, rg * 512:(rg + 1) * 512],
                            start=(dc == 0),
                            stop=(dc == 7),
                        )
                for rg in range(4):
                    pt, off = slices[rg]
                    if bias_sb is not None:
                        nc.vector.tensor_scalar_add(
                            dst_sb[:, rg * 512:(rg + 1) * 512],
                            pt[:, off:off + 512],
                            bias_sb[:, 0:1],
                        )
                    else:
                        nc.vector.tensor_copy(
                            dst_sb[:, rg * 512:(rg + 1) * 512], pt[:, off:off + 512]
                        )

            proj(kt_d, wk_sb, kT_sb, bk_sb)
            proj(qt_d, wq_sb, qT_sb, bq_sb)
            proj(vt_d, wv_sb, vT_sb, None)  # bv folded in after attnV

            # vT -> v natural via PE transpose; split into vaug head slots
            nc.vector.memset(vaug_sb[:, :, 64:65], 1.0)
            nc.vector.memset(vaug_sb[:, :, 129:130], 1.0)
            for rt in range(16):
                tp = pa.tile([128, 128], BF16, tag="pa", name="tp")
                nc.tensor.transpose(tp, vT_sb[:, rt * 128:(rt + 1) * 128], ident)
                nc.vector.tensor_copy(vaug_sb[:, rt, 0:64], tp[:, 0:64])
                nc.vector.tensor_copy(vaug_sb[:, rt, 65:129], tp[:, 64:128])

            # ---- attention (2 heads, full causal triangle) ----
            # probs stay UNNORMALIZED; the vaug ones-column accumulates the
            # softmax denominator into psum row 64, and normalization is one
            # fused [64,128] multiply at evacuation.
            for b in range(16):
                for h in range(2):
                    hp = slice(h * 64, (h + 1) * 64)
                    nkc = b + 1
                    probs = prpool.tile([128, nkc * 128], BF16, tag="probs", name="probs")
                    ngroups = (nkc + 7) // 8
                    for g in range(ngroups):
                        nk_g = min(8, nkc - g * 8)
                        ps = big.tile([128, 1024], F32, tag="big", name="ps")
                        for kci in range(nk_g):
                            kc = g * 8 + kci
                            nc.tensor.matmul(
                                ps[:, kci * 128:(kci + 1) * 128],
                                lhsT=kT_sb[hp, kc * 128:(kc + 1) * 128],
                                rhs=qT_sb[hp, b * 128:(b + 1) * 128],
                                start=True,
                                stop=True,
                            )
                        nc.scalar.activation(
                            probs[:, g * 1024:g * 1024 + nk_g * 128],
                            ps[:, 0:nk_g * 128],
                            Exp,
                            scale=0.125,
                        )
                    # causal mask on the diagonal chunk
                    nc.vector.tensor_mul(
                        probs[:, b * 128:(b + 1) * 128],
                        probs[:, b * 128:(b + 1) * 128],
                        mask_sb,
                    )
                    # attnV+denominator accumulate over kc -> [65, 128]
                    pav = pa.tile([128, 128], F32, tag="pa", name="pav")
                    for kc in range(nkc):
                        nc.tensor.matmul(
                            pav[0:65, :],
                            lhsT=vaug_sb[:, kc, h * 65:(h + 1) * 65],
                            rhs=probs[:, kc * 128:(kc + 1) * 128],
                            start=(kc == 0),
                            stop=(kc == nkc - 1),
                        )
                    # denominator row -> SBUF (ACT), broadcast to 64
                    # partitions (PE), then reciprocal on a full tile (DVE)
                    drow = accpool.tile([1, 128], BF16, tag="drow", name="drow")
                    with nc.allow_low_precision(reason="bf16 softmax denom ok at 2e-2 gate"):
                        nc.scalar.activation(
                            drow, pav[64:65, :],
                            mybir.ActivationFunctionType.Copy,
                        )
                    pbt = pa.tile([128, 128], F32, tag="pa", name="pbt")
                    nc.tensor.matmul(
                        pbt[0:64, :], lhsT=ones_row[:, 0:64], rhs=drow,
                        start=True, stop=True,
                    )
                    bc = accpool.tile([64, 128], BF16, tag="bc", name="bc")
                    with nc.allow_low_precision(reason="bf16 softmax recip ok at 2e-2 gate"):
                        nc.vector.reciprocal(bc, pbt[0:64, :])
                    # fused normalize + evacuate (bv is added after the A2A)
                    nc.vector.tensor_mul(attnT_sb[hp, b, :], pav[0:64, :], bc)

            # ---- AllToAll: head-split -> row-split ----
            a2a_in = dram.tile([1024, 256], BF16, name="a2a_in")
            a2a_out = dram.tile([1024, 256], BF16, name="a2a_out")
            for j in range(8):
                nc.sync.dma_start(
                    out=a2a_in[j * 128:(j + 1) * 128, :],
                    in_=attnT_sb[:, 2 * j:2 * j + 2, :],
                )
            nc.gpsimd.collective_compute(
                "AllToAll",
                mybir.AluOpType.bypass,
                ins=[a2a_in.opt()],
                outs=[a2a_out.opt()],
                replica_groups=[list(range(NCORES))],
            )
            nc.sync.dma_start(
                out=a2a_sb, in_=a2a_out.rearrange("(i p) r -> p i r", p=128)
            )
            # add bv (per-partition = head-dim) post-A2A
            for i in range(8):
                nc.vector.tensor_scalar_add(
                    a2a_sb[:, i, :], a2a_sb[:, i, :], bv_sb[:, i:i + 1]
                )

            # ---- output projection: out.T[m, r] = sum_n WoT[n, m] attnT[n, r] ----
            for t in range(8):
                po = pa.tile([128, 256], F32, tag="pa", name="po")
                for i in range(8):
                    nc.tensor.matmul(
                        po,
                        lhsT=wo_sb[:, i, t * 128:(t + 1) * 128],
                        rhs=a2a_sb[:, i, :],
                        start=(i == 0),
                        stop=(i == 7),
                    )
                nc.vector.tensor_scalar_add(outT_sb[:, t, :], po, bo_sb[:, t:t + 1])
                nc.sync.dma_start(
                    out=out_d[t * 128:(t + 1) * 128, :], in_=outT_sb[:, t, :]
                )

    nc.finalize()
    return nc


def _get_nc():
    if "nc" not in _CACHE:
        _CACHE["nc"] = _build()
    return _CACHE["nc"]


def make_in_maps(Q, K, V, Wq, bq, Wk, bk, Wv, bv, Wo, bo):
    bf16 = ml_dtypes.bfloat16
    f32 = np.float32

    QT = np.ascontiguousarray(np.asarray(Q).T).astype(bf16)  # [D, S]
    KT = np.ascontiguousarray(np.asarray(K).T).astype(bf16)
    VT = np.ascontiguousarray(np.asarray(V).T).astype(bf16)
    WqT = np.ascontiguousarray(np.asarray(Wq).T).astype(bf16)  # [D, D] = [d, m]
    WkT = np.ascontiguousarray(np.asarray(Wk).T).astype(bf16)
    WvT = np.ascontiguousarray(np.asarray(Wv).T).astype(bf16)
    WoT = np.ascontiguousarray(np.asarray(Wo).T).astype(bf16)  # [n, m]
    bq = np.asarray(bq, f32)
    bk = np.asarray(bk, f32)
    bv = np.asarray(bv, f32)
    bo = np.asarray(bo, f32)
    bv_t = np.ascontiguousarray(bv.reshape(8, 128).T).astype(f32)  # [128, 8]
    bo_t = np.ascontiguousarray(bo.reshape(8, 128).T).astype(f32)  # [128, 8]
    mask = np.ascontiguousarray(
        np.triu(np.ones((128, 128), dtype=np.float32))
    ).astype(bf16)  # keep key j <= query i in [j, i] layout

    in_maps = []
    for c in range(NCORES):
        hs = slice(c * 128, (c + 1) * 128)  # this core's 2 heads' proj dims
        in_maps.append(
            {
                "qt": QT,
                "kt": KT,
                "vt": VT,
                "wq": np.ascontiguousarray(WqT[:, hs]),
                "wk": np.ascontiguousarray(WkT[:, hs]),
                "wv": np.ascontiguousarray(WvT[:, hs]),
                "wo": WoT,
                "bq": np.ascontiguousarray(bq[hs].reshape(128, 1)),
                "bk": np.ascontiguousarray(bk[hs].reshape(128, 1)),
                "bv": bv_t,
                "bo": bo_t,
                "mask": mask,
            }
        )
    return in_maps


def kernel(Q, K, V, Wq, bq, Wk, bk, Wv, bv, Wo, bo):
    from concourse.bass_utils import run_bass_kernel_spmd

    in_maps = make_in_maps(Q, K, V, Wq, bq, Wk, bk, Wv, bv, Wo, bo)
    nc = _get_nc()
    res = run_bass_kernel_spmd(nc, in_maps, core_ids=list(range(NCORES)))
    _CACHE["last_results"] = res
    outs = [r["out"] for r in res.results]  # each [D, RPC] f32 = out.T rows chunk
    final = np.concatenate([o.T for o in outs], axis=0)  # [S, D]
    return final.astype(np.float32)


# revision 18
# speedup vs baseline: 1.3949x; 1.1970x over previous
"""Causal MHA (AdaLNAttention) on 8 TRN2 NeuronCores.

Sharding: tensor-parallel on heads (2 heads/core) for QKV projections and
attention (fully local, perfectly balanced since every core does the full
causal triangle for its heads), then one small AllToAll re-shards from
head-split to row-split, and the output projection runs row-parallel
(256 rows/core) with no reduction.

All matmuls run in bf16 (f32 PSUM accumulate); softmax exp in f32 on ACT.
Host-side work is layout/dtype only: transposes, slicing, bf16 casts.
"""

import math

import numpy as np
import ml_dtypes

S = 2048
D = 1024
H = 16
HD = 64
NCORES = 8
RPC = S // NCORES  # 256 rows per core after A2A

_CACHE = {}


def _build():
    import concourse.bass as bass
    import concourse.mybir as mybir
    import concourse.tile as tile
    from concourse import bacc
    from concourse.masks import make_identity

    BF16 = mybir.dt.bfloat16
    F32 = mybir.dt.float32
    Exp = mybir.ActivationFunctionType.Exp

    nc = bacc.Bacc(
        "TRN2", target_bir_lowering=False, debug=False, num_devices=NCORES
    )

    qt_d = nc.declare_dram_parameter("qt", [D, S], BF16, isOutput=False)
    kt_d = nc.declare_dram_parameter("kt", [D, S], BF16, isOutput=False)
    vt_d = nc.declare_dram_parameter("vt", [D, S], BF16, isOutput=False)
    wq_d = nc.declare_dram_parameter("wq", [D, 128], BF16, isOutput=False)
    wk_d = nc.declare_dram_parameter("wk", [D, 128], BF16, isOutput=False)
    wv_d = nc.declare_dram_parameter("wv", [D, 128], BF16, isOutput=False)
    wo_d = nc.declare_dram_parameter("wo", [D, D], BF16, isOutput=False)
    bq_d = nc.declare_dram_parameter("bq", [128, 1], F32, isOutput=False)
    bk_d = nc.declare_dram_parameter("bk", [128, 1], F32, isOutput=False)
    bv_d = nc.declare_dram_parameter("bv", [128, 8], F32, isOutput=False)
    bo_d = nc.declare_dram_parameter("bo", [128, 8], F32, isOutput=False)
    mask_d = nc.declare_dram_parameter("mask", [128, 128], BF16, isOutput=False)
    out_d = nc.declare_dram_parameter("out", [D, RPC], F32, isOutput=True)

    with tile.TileContext(nc) as tc:
        with (
            tc.tile_pool(name="cpool", bufs=1) as cpool,
            tc.tile_pool(name="wpool", bufs=1) as wpool,
            tc.tile_pool(name="spool", bufs=1) as spool,
            tc.tile_pool(name="chunks", bufs=4) as chpool,
            tc.tile_pool(name="probsp", bufs=4) as prpool,
            tc.tile_pool(name="accp", bufs=4) as accpool,
            tc.tile_pool(name="big", bufs=2, space="PSUM") as big,
            tc.tile_pool(name="pa", bufs=4, space="PSUM") as pa,
            tc.tile_pool(name="dram", bufs=1, space="DRAM") as dram,
        ):
            # ---- constants / small inputs ----
            mask_sb = cpool.tile([128, 128], BF16)
            nc.sync.dma_start(out=mask_sb, in_=mask_d[:, :])
            bq_sb = cpool.tile([128, 1], F32)
            nc.sync.dma_start(out=bq_sb, in_=bq_d[:, :])
            bk_sb = cpool.tile([128, 1], F32)
            nc.sync.dma_start(out=bk_sb, in_=bk_d[:, :])
            bv_sb = cpool.tile([128, 8], F32)
            nc.sync.dma_start(out=bv_sb, in_=bv_d[:, :])
            bo_sb = cpool.tile([128, 8], F32)
            nc.sync.dma_start(out=bo_sb, in_=bo_d[:, :])
            ones_row = cpool.tile([1, 128], BF16)
            nc.vector.memset(ones_row, 1.0)
            ident = cpool.tile([128, 128], BF16)
            make_identity(nc, ident)

            # ---- weights ----
            wq_sb = wpool.tile([128, 8, 128], BF16)
            nc.sync.dma_start(out=wq_sb, in_=wq_d.rearrange("(dc p) m -> p dc m", p=128))
            wk_sb = wpool.tile([128, 8, 128], BF16)
            nc.sync.dma_start(out=wk_sb, in_=wk_d.rearrange("(dc p) m -> p dc m", p=128))
            wv_sb = wpool.tile([128, 8, 128], BF16)
            nc.sync.dma_start(out=wv_sb, in_=wv_d.rearrange("(dc p) m -> p dc m", p=128))
            wo_sb = wpool.tile([128, 8, 1024], BF16)
            nc.sync.dma_start(out=wo_sb, in_=wo_d.rearrange("(i p) m -> p i m", p=128))

            # ---- persistent activations ----
            qT_sb = spool.tile([128, S], BF16)   # [2 heads x 64, 2048 rows]
            kT_sb = spool.tile([128, S], BF16)
            vT_sb = spool.tile([128, S], BF16)
            # [keys-in-chunk, kc, (v_h0 | 1 | v_h1 | 1)] — ones col rides the
            # attnV matmul to produce the softmax denominator for free
            vaug_sb = spool.tile([128, 16, 130], BF16)
            attnT_sb = spool.tile([128, 16, 128], BF16)  # [2 heads x 64, block, q-in-block]
            a2a_sb = spool.tile([128, 8, 256], BF16)
            outT_sb = spool.tile([128, 8, 256], F32)

            # ---- projections: out.T[m, r] = sum_d WT[d, m] * XT[d, r] ----
            def proj(in_d, w_sb, dst_sb, bias_sb):
                p0 = big.tile([128, 1024], F32, tag="big", name="p0")
                p1 = big.tile([128, 1024], F32, tag="big", name="p1")
                slices = [(p0, 0), (p0, 512), (p1, 0), (p1, 512)]
                for dc in range(8):
                    ch = chpool.tile([128, S], BF16, tag="chunk", name="ch")
                    nc.sync.dma_start(out=ch, in_=in_d[dc * 128:(dc + 1) * 128, :])
                    for rg in range(4):
                        pt, off = slices[rg]
                        nc.tensor.matmul(
                            pt[:, off:off + 512],
                            lhsT=w_sb[:, dc, :],
                            rhs=ch[:, rg * 512:(rg + 1) * 512],
                            start=(dc == 0),
                            stop=(dc == 7),
                        )
                for rg in range(4):
                    pt, off = slices[rg]
                    if bias_sb is not None:
                        nc.vector.tensor_scalar_add(
                            dst_sb[:, rg * 512:(rg + 1) * 512],
                            pt[:, off:off + 512],
                            bias_sb[:, 0:1],
                        )
                    else:
                        nc.vector.tensor_copy(
                            dst_sb[:, rg * 512:(rg + 1) * 512], pt[:, off:off + 512]
                        )

            proj(kt_d, wk_sb, kT_sb, bk_sb)
            proj(qt_d, wq_sb, qT_sb, bq_sb)
            proj(vt_d, wv_sb, vT_sb, None)  # bv folded in after attnV

            # vT -> v natural via PE transpose; split into vaug head slots
            nc.vector.memset(vaug_sb[:, :, 64:65], 1.0)
            nc.vector.memset(vaug_sb[:, :, 129:130], 1.0)
            for rt in range(16):
                tp = pa.tile([128, 128], BF16, tag="pa", name="tp")
                nc.tensor.transpose(tp, vT_sb[:, rt * 128:(rt + 1) * 128], ident)
                nc.vector.tensor_copy(vaug_sb[:, rt, 0:64], tp[:, 0:64])
                nc.vector.tensor_copy(vaug_sb[:, rt, 65:129], tp[:, 64:128])

            # ---- attention (2 heads, full causal triangle) ----
            # Blocks processed in PAIRS (2B, 2B+1): shared key chunks kc<=2B
            # serve both blocks with one N=256 matmul. probs layout per head:
            # [128, kc-slot, 256] with block 2B in cols 0:128, 2B+1 in
            # 128:256; the final chunk (2B+1, right block only) sits in cols
            # 0:128 of its slot and attnV reads it explicitly. probs stay
            # UNNORMALIZED; the vaug ones-column accumulates the softmax
            # denominator into psum row 64.
            for B in range(8):
                blo = 2 * B
                nsh = blo + 1          # shared chunks 0..2B
                nslots = blo + 2       # + the final right-block-only chunk
                probs01 = [
                    prpool.tile([128, nslots, 256], BF16, tag="probs", name="probs0"),
                    prpool.tile([128, nslots, 256], BF16, tag="probs", name="probs1"),
                ]
                hps = [slice(0, 64), slice(64, 128)]
                ngroups = (nslots + 3) // 4
                for g in range(ngroups):
                    nk_g = min(4, nslots - g * 4)
                    pss = [
                        big.tile([128, 1024], F32, tag="big", name="ps0"),
                        big.tile([128, 1024], F32, tag="big", name="ps1"),
                    ]
                    for kci in range(nk_g):
                        kc = g * 4 + kci
                        # h0/h1 emitted adjacently -> concurrent PE row tiles
                        for h in range(2):
                            if kc < nsh:
                                nc.tensor.matmul(
                                    pss[h][:, kci * 256:(kci + 1) * 256],
                                    lhsT=kT_sb[hps[h], kc * 128:(kc + 1) * 128],
                                    rhs=qT_sb[hps[h], blo * 128:(blo + 2) * 128],
                                    start=True,
                                    stop=True,
                                )
                            else:
                                nc.tensor.matmul(
                                    pss[h][:, kci * 256:kci * 256 + 128],
                                    lhsT=kT_sb[hps[h], kc * 128:(kc + 1) * 128],
                                    rhs=qT_sb[hps[h], (blo + 1) * 128:(blo + 2) * 128],
                                    start=True,
                                    stop=True,
                                )
                    for h in range(2):
                        nc.scalar.activation(
                            probs01[h][:, g * 4:g * 4 + nk_g, :],
                            pss[h][:, 0:nk_g * 256],
                            Exp,
                            scale=0.125,
                        )
                for h in range(2):
                    probs = probs01[h]
                    # causal masks: block 2B diag at slot 2B cols 0:128,
                    # block 2B+1 diag at slot 2B+1 cols 0:128
                    nc.vector.tensor_mul(
                        probs[:, blo, 0:128], probs[:, blo, 0:128], mask_sb
                    )
                    nc.vector.tensor_mul(
                        probs[:, blo + 1, 0:128], probs[:, blo + 1, 0:128], mask_sb
                    )
                for h in range(2):
                    probs = probs01[h]
                    haug = slice(h * 65, (h + 1) * 65)
                    # attnV+denominator accumulate -> [65, 256]
                    pav = pa.tile([128, 256], F32, tag="pa", name="pav")
                    for kc in range(nsh):
                        nc.tensor.matmul(
                            pav[0:65, :],
                            lhsT=vaug_sb[:, kc, haug],
                            rhs=probs[:, kc, :],
                            start=(kc == 0),
                            stop=False,
                            skip_group_check=True,
                        )
                    nc.tensor.matmul(
                        pav[0:65, 128:256],
                        lhsT=vaug_sb[:, nsh, haug],
                        rhs=probs[:, nsh, 0:128],
                        start=False,
                        stop=True,
                        skip_group_check=True,
                    )
                    # denominator row -> SBUF (ACT), broadcast to 64
                    # partitions (PE), then fast approx reciprocal (DVE)
                    drow = accpool.tile([1, 256], BF16, tag="drow", name="drow")
                    with nc.allow_low_precision(reason="bf16 softmax denom ok at 2e-2 gate"):
                        nc.scalar.activation(
                            drow, pav[64:65, :],
                            mybir.ActivationFunctionType.Copy,
                        )
                    pbt = pa.tile([128, 256], F32, tag="pa", name="pbt")
                    nc.tensor.matmul(
                        pbt[0:64, :], lhsT=ones_row[:, 0:64], rhs=drow,
                        start=True, stop=True,
                    )
                    bcf = accpool.tile([64, 256], F32, tag="bcf", name="bcf")
                    nc.vector.reciprocal_approx_fast(bcf, pbt[0:64, :])
                    # fused normalize + evacuate (bv is added after the A2A)
                    nc.vector.tensor_mul(
                        attnT_sb[hps[h], blo:blo + 2, :], pav[0:64, :], bcf
                    )

            # ---- AllToAll: head-split -> row-split ----
            a2a_in = dram.tile([1024, 256], BF16, name="a2a_in")
            a2a_out = dram.tile([1024, 256], BF16, name="a2a_out")
            for j in range(8):
                nc.sync.dma_start(
                    out=a2a_in[j * 128:(j + 1) * 128, :],
                    in_=attnT_sb[:, 2 * j:2 * j + 2, :],
                )
            nc.gpsimd.collective_compute(
                "AllToAll",
                mybir.AluOpType.bypass,
                ins=[a2a_in.opt()],
                outs=[a2a_out.opt()],
                replica_groups=[list(range(NCORES))],
            )
            nc.sync.dma_start(
                out=a2a_sb, in_=a2a_out.rearrange("(i p) r -> p i r", p=128)
            )
            # add bv (per-partition = head-dim) post-A2A
            for i in range(8):
                nc.vector.tensor_scalar_add(
                    a2a_sb[:, i, :], a2a_sb[:, i, :], bv_sb[:, i:i + 1]
                )

            # ---- output projection: out.T[m, r] = sum_n WoT[n, m] attnT[n, r] ----
            for t in range(8):
                po = pa.tile([128, 256], F32, tag="pa", name="po")
                for i in range(8):
                    nc.tensor.matmul(
                        po,
                        lhsT=wo_sb[:, i, t * 128:(t + 1) * 128],
                        rhs=a2a_sb[:, i, :],
                        start=(i == 0),
                        stop=(i == 7),
                    )
                nc.vector.tensor_scalar_add(outT_sb[:, t, :], po, bo_sb[:, t:t + 1])
                nc.sync.dma_start(
                    out=out_d[t * 128:(t + 1) * 128, :], in_=outT_sb[:, t, :]
                )

    nc.finalize()
    return nc


def _get_nc():
    if "nc" not in _CACHE:
        _CACHE["nc"] = _build()
    return _CACHE["nc"]


def make_in_maps(Q, K, V, Wq, bq, Wk, bk, Wv, bv, Wo, bo):
    bf16 = ml_dtypes.bfloat16
    f32 = np.float32

    QT = np.ascontiguousarray(np.asarray(Q).T).astype(bf16)  # [D, S]
    KT = np.ascontiguousarray(np.asarray(K).T).astype(bf16)
    VT = np.ascontiguousarray(np.asarray(V).T).astype(bf16)
    WqT = np.ascontiguousarray(np.asarray(Wq).T).astype(bf16)  # [D, D] = [d, m]
    WkT = np.ascontiguousarray(np.asarray(Wk).T).astype(bf16)
    WvT = np.ascontiguousarray(np.asarray(Wv).T).astype(bf16)
    WoT = np.ascontiguousarray(np.asarray(Wo).T).astype(bf16)  # [n, m]
    bq = np.asarray(bq, f32)
    bk = np.asarray(bk, f32)
    bv = np.asarray(bv, f32)
    bo = np.asarray(bo, f32)
    bv_t = np.ascontiguousarray(bv.reshape(8, 128).T).astype(f32)  # [128, 8]
    bo_t = np.ascontiguousarray(bo.reshape(8, 128).T).astype(f32)  # [128, 8]
    mask = np.ascontiguousarray(
        np.triu(np.ones((128, 128), dtype=np.float32))
    ).astype(bf16)  # keep key j <= query i in [j, i] layout

    in_maps = []
    for c in range(NCORES):
        hs = slice(c * 128, (c + 1) * 128)  # this core's 2 heads' proj dims
        in_maps.append(
            {
                "qt": QT,
                "kt": KT,
                "vt": VT,
                "wq": np.ascontiguousarray(WqT[:, hs]),
                "wk": np.ascontiguousarray(WkT[:, hs]),
                "wv": np.ascontiguousarray(WvT[:, hs]),
                "wo": WoT,
                "bq": np.ascontiguousarray(bq[hs].reshape(128, 1)),
                "bk": np.ascontiguousarray(bk[hs].reshape(128, 1)),
                "bv": bv_t,
                "bo": bo_t,
                "mask": mask,
            }
        )
    return in_maps


def kernel(Q, K, V, Wq, bq, Wk, bk, Wv, bv, Wo, bo):
    from concourse.bass_utils import run_bass_kernel_spmd

    in_maps = make_in_maps(Q, K, V, Wq, bq, Wk, bk, Wv, bv, Wo, bo)
    nc = _get_nc()
    res = run_bass_kernel_spmd(nc, in_maps, core_ids=list(range(NCORES)))
    _CACHE["last_results"] = res
    outs = [r["out"] for r in res.results]  # each [D, RPC] f32 = out.T rows chunk
    final = np.concatenate([o.T for o in outs], axis=0)  # [S, D]
    return final.astype(np.float32)
